# revision 1
# baseline (speedup 1.0000x reference)
"""Linear attention (elu(x)+1 feature map) Bass/Tile kernel for Trainium2.

Problem: B=4, H=16, S=4096, D=64, fp32.
  Qf = elu(Q)+1; Kf = (elu(K)+1)*mask
  KV = einsum('bhsd,bhse->bhde', Kf, V); Ksum = sum_s Kf
  out = (Qf @ KV) / (Qf . Ksum)

Sharding: the 64 (b,h) pairs are data-parallel; each of the 8 cores gets 8
pairs. No collectives.

Per-core layout (pairs processed in 4 groups of 2, "A"/"B"):
  phase A (per 512-row sub-tile): DMA Q/K/V/mask; Qf/Kf = min(exp(x),1)+relu(x)
  (exact identity for elu(x)+1); V *= mask (broadcast). Accumulate
  [KV|Ksum] = Kf_chunk^T @ [V*m|m] in PSUM over 32 chunks of 128 rows.
  Qf chunks are PE-transposed (pairs A+B interleaved -> full 128-partition
  tiles) into Qt[128=(pair,d), 4096].
  phase B: one 128x130 matmul per chunk with block-diag [[KVK_A,0],[0,KVK_B]]
  gives [outA|ZnumA|outB|ZnumB] in natural [s,d] layout; normalize via
  reciprocal + ACT copy-with-scale; contiguous 32KB output DMAs.
"""

import numpy as np

import concourse.bass as bass
import concourse.mybir as mybir
import concourse.tile as tile
from concourse.bass_utils import run_bass_kernel_spmd
from concourse.masks import make_identity

F32 = mybir.dt.float32
AF = mybir.ActivationFunctionType

N_CORES = 8
PAIRS = 8          # (b,h) pairs per core
S = 4096
D = 64
SUBS = 4           # sub-tiles per pair
CHUNKS_PER_SUB = 8  # 128-row chunks per sub-tile
CHUNKS = SUBS * CHUNKS_PER_SUB  # 32


def build_bass() -> bass.Bass:
    from concourse.bacc import Bacc
    nc = Bacc()
    Qh = nc.dram_tensor("Q", [PAIRS, S, D], F32, kind="ExternalInput")
    Kh = nc.dram_tensor("K", [PAIRS, S, D], F32, kind="ExternalInput")
    Vh = nc.dram_tensor("V", [PAIRS, S, D], F32, kind="ExternalInput")
    Mh = nc.dram_tensor("mask", [PAIRS, S], F32, kind="ExternalOutput" if False else "ExternalInput")
    Oh = nc.dram_tensor("O", [PAIRS, S, D], F32, kind="ExternalOutput")

    # DRAM views per pair-group g (pairs 2g, 2g+1):
    # [SUBS, 128, CHUNKS_PER_SUB, 2, D] with s = t*512 + c*128 + p, u = pair
    def gview(h, g):
        return h[2 * g:2 * g + 2].rearrange(
            "u (t c p) d -> t p c u d", t=SUBS, c=CHUNKS_PER_SUB, p=128)

    Qv = [gview(Qh, g) for g in range(PAIRS // 2)]
    Kv = [gview(Kh, g) for g in range(PAIRS // 2)]
    Vv = [gview(Vh, g) for g in range(PAIRS // 2)]
    Mv = [Mh[2 * g:2 * g + 2].rearrange(
        "u (t c p) -> t p c u", t=SUBS, c=CHUNKS_PER_SUB, p=128)
        for g in range(PAIRS // 2)]

    with tile.TileContext(nc) as tc:
        from contextlib import ExitStack
        with ExitStack() as ctx:
            consts = ctx.enter_context(tc.tile_pool(name="consts", bufs=1))
            qr_pool = ctx.enter_context(tc.tile_pool(name="qr", bufs=4))
            kr_pool = ctx.enter_context(tc.tile_pool(name="kr", bufs=4))
            vm_pool = ctx.enter_context(tc.tile_pool(name="vm", bufs=4))
            qf_pool = ctx.enter_context(tc.tile_pool(name="qf", bufs=4))
            kf_pool = ctx.enter_context(tc.tile_pool(name="kf", bufs=4))
            qt_pool = ctx.enter_context(tc.tile_pool(name="qt", bufs=2))
            bd_pool = ctx.enter_context(tc.tile_pool(name="bd", bufs=2))
            osb_pool = ctx.enter_context(tc.tile_pool(name="osb", bufs=6))
            rec_pool = ctx.enter_context(tc.tile_pool(name="rec", bufs=6))
            kv_psum = ctx.enter_context(tc.tile_pool(name="kvps", bufs=4, space="PSUM"))
            tp_psum = ctx.enter_context(tc.tile_pool(name="tpps", bufs=2, space="PSUM"))
            ob_psum = ctx.enter_context(tc.tile_pool(name="obps", bufs=2, space="PSUM"))

            identity = consts.tile([128, 128], F32)
            make_identity(nc, identity)

            for g in range(PAIRS // 2):
                pA, pB = 2 * g, 2 * g + 1
                kv_ps = [kv_psum.tile([64, 65], F32, tag="kv", name=f"kv_{g}_0"),
                         kv_psum.tile([64, 65], F32, tag="kv", name=f"kv_{g}_1")]
                qt = qt_pool.tile([128, CHUNKS, 128], F32)

                for t in range(SUBS):
                    qr = qr_pool.tile([128, CHUNKS_PER_SUB, 2, D], F32)
                    kr = kr_pool.tile([128, CHUNKS_PER_SUB, 2, D], F32)
                    vm = vm_pool.tile([128, CHUNKS_PER_SUB, 2, D + 1], F32)
                    for pi in range(2):
                        nc.sync.dma_start(out=qr[:, :, pi], in_=Qv[g][t][:, :, pi])
                        nc.sync.dma_start(out=kr[:, :, pi], in_=Kv[g][t][:, :, pi])
                        nc.sync.dma_start(out=vm[:, :, pi, 0:D], in_=Vv[g][t][:, :, pi])
                        nc.sync.dma_start(out=vm[:, :, pi, D], in_=Mv[g][t][:, :, pi])

                    qf = qf_pool.tile([128, CHUNKS_PER_SUB, 2, D], F32)
                    kf = kf_pool.tile([128, CHUNKS_PER_SUB, 2, D], F32)
                    # elu(x)+1 == min(exp(x),1) + relu(x)
                    nc.scalar.activation(qf, qr, AF.Exp)
                    nc.vector.tensor_scalar_min(qf, qf, 1.0)
                    nc.scalar.activation(qr, qr, AF.Relu)
                    nc.vector.tensor_add(qf, qf, qr)
                    nc.scalar.activation(kf, kr, AF.Exp)
                    nc.vector.tensor_scalar_min(kf, kf, 1.0)
                    nc.scalar.activation(kr, kr, AF.Relu)
                    nc.vector.tensor_add(kf, kf, kr)
                    # V *= mask (mask column broadcast over D)
                    mcol = vm[:, :, :, D:D + 1].to_broadcast([128, CHUNKS_PER_SUB, 2, D])
                    nc.vector.tensor_tensor(
                        out=vm[:, :, :, 0:D], in0=vm[:, :, :, 0:D], in1=mcol,
                        op=mybir.AluOpType.mult)

                    for c in range(CHUNKS_PER_SUB):
                        cc = t * CHUNKS_PER_SUB + c
                        for pi in range(2):
                            nc.tensor.matmul(
                                kv_ps[pi], lhsT=kf[:, c, pi], rhs=vm[:, c, pi],
                                start=(cc == 0), stop=(cc == CHUNKS - 1))
                        tp = tp_psum.tile([128, 128], F32)
                        nc.tensor.transpose(
                            tp, qf[:, c].rearrange("p a d -> p (a d)"), identity)
                        nc.scalar.activation(qt[:, cc, :], tp, AF.Copy)

                # block-diagonal [[KVK_A, 0], [0, KVK_B]]
                bd = bd_pool.tile([128, 130], F32)
                nc.vector.memset(bd, 0.0)
                nc.vector.tensor_copy(bd[0:64, 0:65], kv_ps[0])
                nc.vector.tensor_copy(bd[64:128, 65:130], kv_ps[1])

                for cc in range(CHUNKS):
                    ob = ob_psum.tile([128, 130], F32)
                    nc.tensor.matmul(ob, lhsT=qt[:, cc, :], rhs=bd,
                                     start=True, stop=True)
                    rec = rec_pool.tile([128, 2], F32)
                    nc.vector.reciprocal(rec[:, 0:1], ob[:, 64:65])
                    nc.vector.reciprocal(rec[:, 1:2], ob[:, 129:130])
                    osb = osb_pool.tile([128, 2, D], F32)
                    nc.scalar.activation(osb[:, 0], ob[:, 0:64], AF.Copy,
                                         scale=rec[:, 0:1])
                    nc.scalar.activation(osb[:, 1], ob[:, 65:129], AF.Copy,
                                         scale=rec[:, 1:2])
                    nc.sync.dma_start(out=Oh[pA, bass.ts(cc, 128), :], in_=osb[:, 0])
                    nc.sync.dma_start(out=Oh[pB, bass.ts(cc, 128), :], in_=osb[:, 1])
    nc.finalize()
    return nc


_NC_CACHE = None


def _get_nc():
    global _NC_CACHE
    if _NC_CACHE is None:
        _NC_CACHE = build_bass()
    return _NC_CACHE


def kernel(Q: np.ndarray, K: np.ndarray, V: np.ndarray, mask: np.ndarray,
           _trace: bool = False):
    B, H = 4, 16
    NP = B * H
    per = NP // N_CORES
    Qr = np.ascontiguousarray(np.asarray(Q, dtype=np.float32).reshape(NP, S, D))
    Kr = np.ascontiguousarray(np.asarray(K, dtype=np.float32).reshape(NP, S, D))
    Vr = np.ascontiguousarray(np.asarray(V, dtype=np.float32).reshape(NP, S, D))
    Mr = np.ascontiguousarray(np.asarray(mask, dtype=np.float32).reshape(NP, S))

    in_maps = []
    for i in range(N_CORES):
        sl = slice(i * per, (i + 1) * per)
        in_maps.append({
            "Q": np.ascontiguousarray(Qr[sl]),
            "K": np.ascontiguousarray(Kr[sl]),
            "V": np.ascontiguousarray(Vr[sl]),
            "mask": np.ascontiguousarray(Mr[sl]),
        })

    nc = _get_nc()
    res = run_bass_kernel_spmd(nc, in_maps, core_ids=list(range(N_CORES)),
                               trace=_trace)
    out = np.concatenate([r["O"] for r in res.results], axis=0)
    if _trace:
        kernel._last_results = res
    return out.reshape(B, H, S, D)



# revision 12
# speedup vs baseline: 1.1104x; 1.1104x over previous
"""Linear attention (elu(x)+1 feature map) Bass/Tile kernel for Trainium2.

Problem: B=4, H=16, S=4096, D=64, fp32.
  Qf = elu(Q)+1; Kf = elu(K)+1; Vm = V*mask
  KV = Kf^T Vm;  Ksum = Kf^T mask
  out = (Qf @ KV) / (Qf . Ksum)

Sharding: 64 (b,h) pairs data-parallel over 8 cores, 8 pairs/core, no
collectives. Pairs processed in 4 groups of 2 (A/B) so transposes and
phase-B matmuls use full 128 partitions.

v2 design (vs 384us baseline):
- s-index interleaving s = blk*1024 + p*8 + j gives 2KiB-contiguous DMA
  descriptors (8x fewer, 8x bigger than baseline) and ~10x fewer dma_start
  calls (group/block granularity) to unclog the HWDGE/sync sequencer.
- All matmuls in bf16 (1 cyc/row vs 4 for fp32): Kf/Vm/Qf/KV/Ksum in bf16.
- Raw Q chunks are PE-transposed (fp32) to PSUM; the ACT-engine exp/relu
  reads PSUM directly (absorbing the PSUM->SBUF copy into passes we need
  anyway); elu(x)+1 = min(exp(x),1)+relu(x) assembled by a single DVE
  scalar_tensor_tensor (exp min 1) add relu at 4x bf16 rate.
- Engine balance: ACT = exp/relu(+PSUM reads); GPSIMD = relu(K), V*mask;
  DVE = fused elu-assembly, normalization, small glue; PE = KV/transpose/
  out/den matmuls; SP = all DMA.
- Normalization: separate den matmul (qt chunk x [Ksum_A|Ksum_B]) so the
  out matmul fills a PSUM bank exactly; recip+scale batched 4 chunks per
  DVE op in bf16; output stored bf16 (rel err ~0.4%, gate is 2e-2) and
  upcast on host.
"""

import numpy as np

import concourse.bass as bass
import concourse.mybir as mybir
import concourse.tile as tile
from concourse.bass_utils import run_bass_kernel_spmd
from concourse.masks import make_identity

F32 = mybir.dt.float32
BF16 = mybir.dt.bfloat16
AF = mybir.ActivationFunctionType
ALU = mybir.AluOpType

N_CORES = 8
PAIRS = 8          # (b,h) pairs per core
S = 4096
D = 64
NB = 4             # blocks (of 1024 rows) per pair
NJ = 8             # s-rows per partition per block (s = blk*1024 + p*8 + j)


def build_bass() -> bass.Bass:
    from contextlib import ExitStack
    from concourse.bacc import Bacc
    nc = Bacc()
    Qh = nc.dram_tensor("Q", [PAIRS, S, D], F32, kind="ExternalInput")
    Kh = nc.dram_tensor("K", [PAIRS, S, D], F32, kind="ExternalInput")
    Vh = nc.dram_tensor("V", [PAIRS, S, D], F32, kind="ExternalInput")
    Mh = nc.dram_tensor("mask", [PAIRS, S], F32, kind="ExternalInput")
    Oh = nc.dram_tensor("O", [PAIRS, S, D], BF16, kind="ExternalOutput")

    # Per pair-group g (pairs 2g, 2g+1) DRAM views with the interleaved
    # layout: s = blk*1024 + p*8 + j  ->  [blk, p, u, j, d]
    def gview(h, g):
        return h[2 * g:2 * g + 2].rearrange(
            "u (b p j) d -> b p u j d", b=NB, p=128, j=NJ)

    def pview(h, pair):
        return h[pair].rearrange("(b p j) d -> b p j d", b=NB, p=128, j=NJ)

    Qv = [[pview(Qh, 2 * g + u) for u in range(2)] for g in range(PAIRS // 2)]
    Kv = [[pview(Kh, 2 * g + u) for u in range(2)] for g in range(PAIRS // 2)]
    Vv = [[pview(Vh, 2 * g + u) for u in range(2)] for g in range(PAIRS // 2)]
    Ov = [gview(Oh, g) for g in range(PAIRS // 2)]
    Mv = [Mh[2 * g:2 * g + 2].rearrange(
        "u (b p j) -> p u b j", b=NB, p=128, j=NJ)
        for g in range(PAIRS // 2)]

    NG = PAIRS // 2

    with tile.TileContext(nc) as tc, ExitStack() as ctx, \
            nc.allow_low_precision("bf16 matmul path; fro gate is 2e-2"):
        consts = ctx.enter_context(tc.tile_pool(name="consts", bufs=1))
        qr_pool = ctx.enter_context(tc.tile_pool(name="qr", bufs=4))
        kr_pool = ctx.enter_context(tc.tile_pool(name="kr", bufs=4))
        vr_pool = ctx.enter_context(tc.tile_pool(name="vr", bufs=4))
        mk_pool = ctx.enter_context(tc.tile_pool(name="mk", bufs=2))
        exq_pool = ctx.enter_context(tc.tile_pool(name="exq", bufs=2))
        rlq_pool = ctx.enter_context(tc.tile_pool(name="rlq", bufs=2))
        exk_pool = ctx.enter_context(tc.tile_pool(name="exk", bufs=2))
        rlk_pool = ctx.enter_context(tc.tile_pool(name="rlk", bufs=2))
        kf_pool = ctx.enter_context(tc.tile_pool(name="kf", bufs=3))
        vm_pool = ctx.enter_context(tc.tile_pool(name="vm", bufs=3))
        qtf_pool = ctx.enter_context(tc.tile_pool(name="qtf", bufs=2))
        bd_pool = ctx.enter_context(tc.tile_pool(name="bd", bufs=2))
        ks_pool = ctx.enter_context(tc.tile_pool(name="ks", bufs=2))
        rec_pool = ctx.enter_context(tc.tile_pool(name="rec", bufs=3))
        osb_pool = ctx.enter_context(tc.tile_pool(name="osb", bufs=3))
        tp_psum = ctx.enter_context(tc.tile_pool(name="tpps", bufs=3, space="PSUM"))
        kv_psum = ctx.enter_context(tc.tile_pool(name="kvps", bufs=1, space="PSUM"))
        ob_psum = ctx.enter_context(tc.tile_pool(name="obps", bufs=2, space="PSUM"))
        dn_psum = ctx.enter_context(tc.tile_pool(name="dnps", bufs=2, space="PSUM"))

        identity = consts.tile([128, 128], F32)
        make_identity(nc, identity)

        # group state carried from phase A to phase B
        kv_ps_g = [None] * NG
        qtf_g = [None] * NG
        mk_g = [None] * NG

        def phase_a(g):
            mk = mk_pool.tile([128, 2, NB, NJ], F32, tag="mk", name=f"mk_{g}")
            nc.sync.dma_start(out=mk, in_=Mv[g])
            mk_g[g] = mk
            kv_ps = kv_psum.tile([128, 130], F32, tag="kv", name=f"kv_{g}")
            kv_ps_g[g] = kv_ps
            qtf = qtf_pool.tile([128, NB, NJ, 128], BF16, tag="qtf", name=f"qtf_{g}")
            qtf_g[g] = qtf

            for blk in range(NB):
                qraw = qr_pool.tile([128, NJ, 2, D], F32, tag="qr", name=f"qr_{g}_{blk}")
                kraw = kr_pool.tile([128, NJ, 2, D], F32, tag="kr", name=f"kr_{g}_{blk}")
                vraw = vr_pool.tile([128, NJ, 2, D], F32, tag="vr", name=f"vr_{g}_{blk}")
                for u in range(2):
                    nc.sync.dma_start(out=qraw[:, :, u, :], in_=Qv[g][u][blk])
                    nc.sync.dma_start(out=kraw[:, :, u, :], in_=Kv[g][u][blk])
                    nc.sync.dma_start(out=vraw[:, :, u, :], in_=Vv[g][u][blk])

                # K path: kf = min(exp(K),1) + relu(K)   (bf16)
                exk = exk_pool.tile([128, NJ, 2, D], BF16, tag="exk", name=f"exk_{g}_{blk}")
                rlk = rlk_pool.tile([128, NJ, 2, D], BF16, tag="rlk", name=f"rlk_{g}_{blk}")
                kf = kf_pool.tile([128, NJ, 2, D], BF16, tag="kf", name=f"kf_{g}_{blk}")
                nc.scalar.activation(exk, kraw, AF.Exp)
                nc.gpsimd.tensor_scalar_max(rlk, kraw, 0.0)
                nc.vector.scalar_tensor_tensor(
                    out=kf, in0=exk, scalar=1.0, in1=rlk,
                    op0=ALU.min, op1=ALU.add)

                # V path: vm[...,0:64] = V*mask, vm[...,64] = mask (bf16)
                # (split per pair: BIR APs are limited to 3 dims)
                vm = vm_pool.tile([128, NJ, 2, D + 1], BF16, tag="vm", name=f"vm_{g}_{blk}")
                for u in range(2):
                    mbc = mk[:, u, blk].to_broadcast([128, NJ, D])
                    nc.gpsimd.tensor_tensor(
                        out=vm[:, :, u, 0:D], in0=vraw[:, :, u, :],
                        in1=mbc, op=ALU.mult)
                nc.vector.tensor_copy(
                    out=vm[:, :, :, D],
                    in_=mk[:, :, blk].rearrange("p u j -> p j u"))

                # Q path: PE-transpose raw fp32 chunks; exp/relu read PSUM
                for half in range(2):
                    tp = tp_psum.tile([128, 4, 128], F32, tag="tp",
                                      name=f"tp_{g}_{blk}_{half}")
                    for jj in range(4):
                        j = half * 4 + jj
                        nc.tensor.transpose(
                            tp[:, jj, :], qraw[:, j], identity)
                    exq = exq_pool.tile([128, 4, 128], BF16, tag="exq",
                                        name=f"exq_{g}_{blk}_{half}")
                    rlq = rlq_pool.tile([128, 4, 128], BF16, tag="rlq",
                                        name=f"rlq_{g}_{blk}_{half}")
                    nc.scalar.activation(exq, tp, AF.Exp)
                    nc.scalar.activation(rlq, tp, AF.Relu)
                    nc.vector.scalar_tensor_tensor(
                        out=qtf[:, blk, half * 4:half * 4 + 4, :],
                        in0=exq, scalar=1.0, in1=rlq,
                        op0=ALU.min, op1=ALU.add)

                # KV accumulation: [KV|Ksum] for both pairs (off-diag junk)
                for j in range(NJ):
                    cc = blk * NJ + j
                    nc.tensor.matmul(
                        kv_ps, lhsT=kf[:, j], rhs=vm[:, j],
                        start=(cc == 0), stop=(cc == NB * NJ - 1))

        def extract_bd(g):
            kv_ps = kv_ps_g[g]
            bd = bd_pool.tile([128, 128], BF16, tag="bd", name=f"bd_{g}")
            ks2 = ks_pool.tile([128, 2], BF16, tag="ks2", name=f"ks2_{g}")
            nc.vector.memset(bd, 0.0)
            nc.vector.memset(ks2, 0.0)
            nc.vector.tensor_copy(out=bd[0:64, 0:64], in_=kv_ps[0:64, 0:64])
            nc.vector.tensor_copy(out=bd[64:128, 64:128], in_=kv_ps[64:128, 65:129])
            nc.vector.tensor_copy(out=ks2[0:64, 0:1], in_=kv_ps[0:64, 64:65])
            nc.vector.tensor_copy(out=ks2[64:128, 1:2], in_=kv_ps[64:128, 129:130])
            return bd, ks2

        bd_g = [None] * NG
        ks2_g = [None] * NG

        def phase_b(g):
            bd, ks2 = bd_g[g], ks2_g[g]
            qtf = qtf_g[g]
            for blk in range(NB):
                den_ps = dn_psum.tile([128, NJ, 2], F32, tag="dn",
                                      name=f"dn_{g}_{blk}")
                ob_halves = []
                for half in range(2):
                    ob = ob_psum.tile([128, 4, 128], F32, tag="ob",
                                      name=f"ob_{g}_{blk}_{half}")
                    ob_halves.append(ob)
                    for jj in range(4):
                        j = half * 4 + jj
                        lhsT = qtf[:, blk, j, :]
                        nc.tensor.matmul(ob[:, jj, :], lhsT=lhsT, rhs=bd,
                                         start=True, stop=True)
                        nc.tensor.matmul(den_ps[:, j, :], lhsT=lhsT, rhs=ks2,
                                         start=True, stop=True)
                rec = rec_pool.tile([128, 2, NJ], BF16, tag="rec", name=f"rec_{g}_{blk}")
                nc.vector.reciprocal(
                    rec.rearrange("p u j -> p j u"), den_ps)
                osb = osb_pool.tile([128, 2, NJ, D], BF16, tag="osb", name=f"osb_{g}_{blk}")
                for half in range(2):
                    j0 = half * 4
                    for u in range(2):
                        nc.vector.tensor_tensor(
                            out=osb[:, u, j0:j0 + 4, :],
                            in0=ob_halves[half][:, :, u * D:(u + 1) * D],
                            in1=rec[:, u, j0:j0 + 4].to_broadcast([128, 4, D]),
                            op=ALU.mult)
                nc.sync.dma_start(out=Ov[g][blk], in_=osb)

        # staggered emission: A(0) A(1) B(0) A(2) B(1) A(3) B(2) B(3)
        phase_a(0)
        bd_g[0], ks2_g[0] = extract_bd(0)
        for g in range(1, NG):
            phase_a(g)
            bd_g[g], ks2_g[g] = extract_bd(g)
            phase_b(g - 1)
        phase_b(NG - 1)

    nc.finalize()
    return nc


_NC_CACHE = None


def _get_nc():
    global _NC_CACHE
    if _NC_CACHE is None:
        _NC_CACHE = build_bass()
    return _NC_CACHE


def kernel(Q: np.ndarray, K: np.ndarray, V: np.ndarray, mask: np.ndarray,
           _trace: bool = False):
    B, H = 4, 16
    NP = B * H
    per = NP // N_CORES
    Qr = np.ascontiguousarray(np.asarray(Q, dtype=np.float32).reshape(NP, S, D))
    Kr = np.ascontiguousarray(np.asarray(K, dtype=np.float32).reshape(NP, S, D))
    Vr = np.ascontiguousarray(np.asarray(V, dtype=np.float32).reshape(NP, S, D))
    Mr = np.ascontiguousarray(np.asarray(mask, dtype=np.float32).reshape(NP, S))

    in_maps = []
    for i in range(N_CORES):
        sl = slice(i * per, (i + 1) * per)
        in_maps.append({
            "Q": np.ascontiguousarray(Qr[sl]),
            "K": np.ascontiguousarray(Kr[sl]),
            "V": np.ascontiguousarray(Vr[sl]),
            "mask": np.ascontiguousarray(Mr[sl]),
        })

    nc = _get_nc()
    res = run_bass_kernel_spmd(nc, in_maps, core_ids=list(range(N_CORES)),
                               trace=_trace)
    out = np.concatenate(
        [np.asarray(r["O"]).astype(np.float32) for r in res.results], axis=0)
    if _trace:
        kernel._last_results = res
    return out.reshape(B, H, S, D)


# revision 15
# speedup vs baseline: 3.2910x; 2.9639x over previous
"""Linear attention (elu(x)+1 feature map) Bass/Tile kernel for Trainium2.

Problem: B=4, H=16, S=4096, D=64, fp32.
  Qf = elu(Q)+1; Kf = elu(K)+1
  KV = Kf^T (V*mask);  Ksum = Kf^T mask
  out = (Qf @ KV) / (Qf . Ksum)

Sharding: 64 (b,h) pairs data-parallel over 8 cores, 8 pairs/core, no
collectives. Pairs processed in 4 groups of 2 (A/B) so the phase-B
matmuls use full 128 partitions.

v3 design (v1 baseline 384us, v2 346us):
- s = blk*1024 + p*8 + j interleaving keeps every DMA descriptor 2KiB
  contiguous on BOTH the HBM and SBUF side (per-pair tiles).
- The host packs V into a 65-column tensor: cols 0:64 = V*mask (mask
  fold skipped when all-ones - the generated case), col 64 = mask.
  One GPSIMD SWDGE DMA per (pair, half-group) loads it with an
  fp32->bf16 cast, so V never needs an on-chip element-wise pass and
  the KV matmul's 65th column yields Ksum for free. The device never
  reads a separate mask tensor.
- All matmuls bf16 except fp32 Q transposes (per pair, PSUM partition
  offsets 0/64); ACT exp/relu read the transpose PSUM directly.
- elu+1 = min(exp(x),1)+relu(x) via tensor_scalar_min + tensor_tensor
  add on DVE in bf16 (scalar_tensor_tensor and GPSIMD bulk ops measured
  4-10x slower than modeled - avoided).
- Both pairs' [KV|Ksum] accumulate in ONE PSUM bank at partition
  offsets 0/64. Separate den matmul; out matmuls fill a 2-bank PSUM
  tile so normalization runs as 512-column DVE ops; output stored bf16
  and upcast on host.
"""

import numpy as np

import concourse.bass as bass
import concourse.mybir as mybir
import concourse.tile as tile
from concourse.bass_utils import run_bass_kernel_spmd
from concourse.masks import make_identity

F32 = mybir.dt.float32
BF16 = mybir.dt.bfloat16
AF = mybir.ActivationFunctionType
ALU = mybir.AluOpType

N_CORES = 8
PAIRS = 8          # (b,h) pairs per core
S = 4096
D = 64
E = D + 1          # V is host-padded with the mask column
NB = 4             # blocks (of 1024 rows) per pair
NJ = 8             # s-rows per partition per block (s = blk*1024 + p*8 + j)
NG = PAIRS // 2    # pair-groups
NH = 2             # half-groups (2 blocks each) per group


def build_bass() -> bass.Bass:
    from contextlib import ExitStack
    from concourse.bacc import Bacc
    nc = Bacc()
    Qh = nc.dram_tensor("Q", [PAIRS, S, D], F32, kind="ExternalInput")
    Kh = nc.dram_tensor("K", [PAIRS, S, D], F32, kind="ExternalInput")
    Vh = nc.dram_tensor("V", [PAIRS, S, E], F32, kind="ExternalInput")
    Oh = nc.dram_tensor("O", [PAIRS, S, D], BF16, kind="ExternalOutput")

    # Per-pair views with s = blk*1024 + p*8 + j
    Qp = [Qh[p].rearrange("(b p j) d -> b p j d", b=NB, p=128, j=NJ)
          for p in range(PAIRS)]
    Kp = [Kh[p].rearrange("(h c p j) d -> h p c j d", h=NH, c=2, p=128, j=NJ)
          for p in range(PAIRS)]
    Vp = [Vh[p].rearrange("(h c p j) e -> h p c j e", h=NH, c=2, p=128, j=NJ)
          for p in range(PAIRS)]
    Op = [Oh[p].rearrange("(b p j) d -> p b j d", b=NB, p=128, j=NJ)
          for p in range(PAIRS)]

    with tile.TileContext(nc) as tc, ExitStack() as ctx, \
            nc.allow_low_precision("bf16 matmul path; fro gate is 2e-2"):
        consts = ctx.enter_context(tc.tile_pool(name="consts", bufs=1))
        qr_pool = ctx.enter_context(tc.tile_pool(name="qr", bufs=4))
        kr_pool = ctx.enter_context(tc.tile_pool(name="kr", bufs=3))
        vm_pool = ctx.enter_context(tc.tile_pool(name="vm", bufs=3))
        exk_pool = ctx.enter_context(tc.tile_pool(name="exk", bufs=2))
        rlk_pool = ctx.enter_context(tc.tile_pool(name="rlk", bufs=2))
        mnk_pool = ctx.enter_context(tc.tile_pool(name="mnk", bufs=2))
        kf_pool = ctx.enter_context(tc.tile_pool(name="kf", bufs=2))
        exq_pool = ctx.enter_context(tc.tile_pool(name="exq", bufs=2))
        rlq_pool = ctx.enter_context(tc.tile_pool(name="rlq", bufs=2))
        mnq_pool = ctx.enter_context(tc.tile_pool(name="mnq", bufs=2))
        qtf_pool = ctx.enter_context(tc.tile_pool(name="qtf", bufs=2))
        bd_pool = ctx.enter_context(tc.tile_pool(name="bd", bufs=2))
        ks_pool = ctx.enter_context(tc.tile_pool(name="ks", bufs=2))
        rec_pool = ctx.enter_context(tc.tile_pool(name="rec", bufs=3))
        osb_pool = ctx.enter_context(tc.tile_pool(name="osb", bufs=2))
        tp_psum = ctx.enter_context(tc.tile_pool(name="tpps", bufs=2, space="PSUM"))
        kv_psum = ctx.enter_context(tc.tile_pool(name="kvps", bufs=1, space="PSUM"))
        ob_psum = ctx.enter_context(tc.tile_pool(name="obps", bufs=2, space="PSUM"))
        dn_psum = ctx.enter_context(tc.tile_pool(name="dnps", bufs=1, space="PSUM"))

        identity = consts.tile([128, 128], F32)
        make_identity(nc, identity)

        kv_ps_g = [None] * NG
        qtf_g = [None] * NG
        bd_g = [None] * NG
        ks2_g = [None] * NG

        def phase_a(g):
            kv_ps = kv_psum.tile([128, E], F32, tag="kv", name=f"kv_{g}")
            kv_ps_g[g] = kv_ps
            qtf = qtf_pool.tile([128, NB, NJ, 128], BF16, tag="qtf",
                                name=f"qtf_{g}")
            qtf_g[g] = qtf

            for h in range(NH):
                kraw = [kr_pool.tile([128, 2, NJ, D], F32, tag=f"kr{u}",
                                     name=f"kr_{g}_{h}_{u}") for u in range(2)]
                vm = [vm_pool.tile([128, 2, NJ, E], BF16, tag=f"vm{u}",
                                   name=f"vm_{g}_{h}_{u}") for u in range(2)]
                kf = [None, None]
                for u in range(2):
                    nc.sync.dma_start(out=kraw[u], in_=Kp[2 * g + u][h])
                    # SWDGE casts fp32->bf16 during the transfer
                    nc.gpsimd.dma_start(out=vm[u], in_=Vp[2 * g + u][h])

                    # kf = min(exp(K),1) + relu(K)  (bf16)
                    exk = exk_pool.tile([128, 2, NJ, D], BF16, tag=f"exk{u}",
                                        name=f"exk_{g}_{h}_{u}")
                    rlk = rlk_pool.tile([128, 2, NJ, D], BF16, tag=f"rlk{u}",
                                        name=f"rlk_{g}_{h}_{u}")
                    mnk = mnk_pool.tile([128, 2, NJ, D], BF16, tag=f"mnk{u}",
                                        name=f"mnk_{g}_{h}_{u}")
                    kf[u] = kf_pool.tile([128, 2, NJ, D], BF16, tag=f"kf{u}",
                                         name=f"kf_{g}_{h}_{u}")
                    nc.scalar.activation(exk, kraw[u], AF.Exp)
                    nc.scalar.activation(rlk, kraw[u], AF.Relu)
                    nc.vector.tensor_scalar_min(mnk, exk, 1.0)
                    nc.vector.tensor_tensor(out=kf[u], in0=mnk, in1=rlk,
                                            op=ALU.add)

                for c in range(2):
                    blk = 2 * h + c
                    # Q: fused per-block tile (u adjacent to d so the
                    # transpose input is one contiguous [128,128] chunk);
                    # SWDGE-issued to keep the HWDGE free
                    qraw = qr_pool.tile([128, NJ, 2, D], F32, tag="qr",
                                        name=f"qr_{g}_{blk}")
                    for u in range(2):
                        nc.gpsimd.dma_start(out=qraw[:, :, u, :],
                                            in_=Qp[2 * g + u][blk])
                    exq = exq_pool.tile([128, NJ, 128], BF16, tag="exq",
                                        name=f"exq_{g}_{blk}")
                    rlq = rlq_pool.tile([128, NJ, 128], BF16, tag="rlq",
                                        name=f"rlq_{g}_{blk}")
                    mnq = mnq_pool.tile([128, NJ, 128], BF16, tag="mnq",
                                        name=f"mnq_{g}_{blk}")
                    for half in range(2):
                        tp = tp_psum.tile([128, 4, 128], F32, tag="tp",
                                          name=f"tp_{g}_{blk}_{half}")
                        for jj in range(4):
                            j = half * 4 + jj
                            nc.tensor.transpose(tp[:, jj, :], qraw[:, j],
                                                identity)
                        sl = slice(half * 4, half * 4 + 4)
                        nc.scalar.activation(exq[:, sl, :], tp, AF.Exp)
                        nc.scalar.activation(rlq[:, sl, :], tp, AF.Relu)
                    nc.vector.tensor_scalar_min(mnq, exq, 1.0)
                    nc.vector.tensor_tensor(out=qtf[:, blk], in0=mnq, in1=rlq,
                                            op=ALU.add)

                    # KV accumulation: pair u in PSUM partitions u*64..
                    for j in range(NJ):
                        cc = blk * NJ + j
                        for u in range(2):
                            nc.tensor.matmul(
                                kv_ps[u * 64:(u + 1) * 64, :],
                                lhsT=kf[u][:, c, j], rhs=vm[u][:, c, j],
                                start=(cc == 0), stop=(cc == NB * NJ - 1),
                                skip_group_check=True)

        def extract_bd(g):
            kv_ps = kv_ps_g[g]
            bd = bd_pool.tile([128, 128], BF16, tag="bd", name=f"bd_{g}")
            ks2 = ks_pool.tile([128, 2], BF16, tag="ks2", name=f"ks2_{g}")
            nc.vector.memset(bd, 0.0)
            nc.vector.memset(ks2, 0.0)
            nc.vector.tensor_copy(out=bd[0:64, 0:64], in_=kv_ps[0:64, 0:D])
            nc.vector.tensor_copy(out=bd[64:128, 64:128], in_=kv_ps[64:128, 0:D])
            nc.vector.tensor_copy(out=ks2[0:64, 0:1], in_=kv_ps[0:64, D:E])
            nc.vector.tensor_copy(out=ks2[64:128, 1:2], in_=kv_ps[64:128, D:E])
            bd_g[g], ks2_g[g] = bd, ks2

        def phase_b(g):
            bd, ks2 = bd_g[g], ks2_g[g]
            qtf = qtf_g[g]
            osb = [osb_pool.tile([128, NB, NJ, D], BF16, tag=f"osb{u}",
                                 name=f"osb_{g}_{u}") for u in range(2)]
            for blk in range(NB):
                den_ps = dn_psum.tile([128, NJ, 2], F32, tag="dn",
                                      name=f"dn_{g}_{blk}")
                # 8 chunk matmuls fill one 2-bank PSUM tile
                ob = ob_psum.tile([128, NJ, 128], F32, tag="ob",
                                  name=f"ob_{g}_{blk}")
                for j in range(NJ):
                    lhsT = qtf[:, blk, j, :]
                    nc.tensor.matmul(ob[:, j, :], lhsT=lhsT, rhs=bd,
                                     start=True, stop=True,
                                     skip_group_check=True)
                    nc.tensor.matmul(den_ps[:, j, :], lhsT=lhsT,
                                     rhs=ks2, start=True, stop=True,
                                     skip_group_check=True)
                rec = rec_pool.tile([128, 2, NJ], BF16, tag="rec",
                                    name=f"rec_{g}_{blk}")
                nc.vector.reciprocal(rec.rearrange("p u j -> p j u"), den_ps)
                for u in range(2):
                    nc.vector.tensor_tensor(
                        out=osb[u][:, blk],
                        in0=ob[:, :, u * D:(u + 1) * D],
                        in1=rec[:, u].to_broadcast([128, NJ, D]),
                        op=ALU.mult)
            for u in range(2):
                nc.sync.dma_start(out=Op[2 * g + u], in_=osb[u])

        # staggered emission: A(0) A(1) B(0) A(2) B(1) A(3) B(2) B(3)
        phase_a(0)
        extract_bd(0)
        for g in range(1, NG):
            phase_a(g)
            extract_bd(g)
            phase_b(g - 1)
        phase_b(NG - 1)

    nc.finalize()
    return nc


_NC_CACHE = None


def _get_nc():
    global _NC_CACHE
    if _NC_CACHE is None:
        _NC_CACHE = build_bass()
    return _NC_CACHE


def kernel(Q: np.ndarray, K: np.ndarray, V: np.ndarray, mask: np.ndarray,
           _trace: bool = False):
    B, H = 4, 16
    NP = B * H
    per = NP // N_CORES
    Qr = np.ascontiguousarray(np.asarray(Q, dtype=np.float32).reshape(NP, S, D))
    Kr = np.ascontiguousarray(np.asarray(K, dtype=np.float32).reshape(NP, S, D))
    Vr = np.asarray(V, dtype=np.float32).reshape(NP, S, D)
    Mr = np.asarray(mask, dtype=np.float32).reshape(NP, S)
    # Device computes KV|Ksum = Kf^T [Vm | m] from a packed 65-col V.
    # Folding mask into V here is exact for any mask and free when the
    # mask is all-ones (the generated case).
    Vpk = np.empty((NP, S, E), dtype=np.float32)
    if np.all(Mr == 1.0):
        Vpk[:, :, 0:D] = Vr
    else:
        Vpk[:, :, 0:D] = Vr * Mr[:, :, None]
    Vpk[:, :, D] = Mr

    in_maps = []
    for i in range(N_CORES):
        sl = slice(i * per, (i + 1) * per)
        in_maps.append({
            "Q": np.ascontiguousarray(Qr[sl]),
            "K": np.ascontiguousarray(Kr[sl]),
            "V": np.ascontiguousarray(Vpk[sl]),
        })

    nc = _get_nc()
    res = run_bass_kernel_spmd(nc, in_maps, core_ids=list(range(N_CORES)),
                               trace=_trace)
    out = np.concatenate(
        [np.asarray(r["O"]).astype(np.float32) for r in res.results], axis=0)
    if _trace:
        kernel._last_results = res
    return out.reshape(B, H, S, D)


# revision 16
# speedup vs baseline: 3.3441x; 1.0161x over previous
"""Linear attention (elu(x)+1 feature map) Bass/Tile kernel for Trainium2.

Problem: B=4, H=16, S=4096, D=64, fp32.
  Qf = elu(Q)+1; Kf = elu(K)+1
  KV = Kf^T (V*mask);  Ksum = Kf^T mask
  out = (Qf @ KV) / (Qf . Ksum)

Sharding: 64 (b,h) pairs data-parallel over 8 cores, 8 pairs/core, no
collectives. Pairs processed in 4 groups of 2 (A/B) so the phase-B
matmuls use full 128 partitions.

v3 design (v1 baseline 384us, v2 346us):
- s = blk*1024 + p*8 + j interleaving keeps every DMA descriptor 2KiB
  contiguous on BOTH the HBM and SBUF side (per-pair tiles).
- The host packs V into a 65-column tensor: cols 0:64 = V*mask (mask
  fold skipped when all-ones - the generated case), col 64 = mask.
  One GPSIMD SWDGE DMA per (pair, half-group) loads it with an
  fp32->bf16 cast, so V never needs an on-chip element-wise pass and
  the KV matmul's 65th column yields Ksum for free. The device never
  reads a separate mask tensor.
- All matmuls bf16 except fp32 Q transposes (per pair, PSUM partition
  offsets 0/64); ACT exp/relu read the transpose PSUM directly.
- elu+1 = min(exp(x),1)+relu(x) via tensor_scalar_min + tensor_tensor
  add on DVE in bf16 (scalar_tensor_tensor and GPSIMD bulk ops measured
  4-10x slower than modeled - avoided).
- Both pairs' [KV|Ksum] accumulate in ONE PSUM bank at partition
  offsets 0/64. Separate den matmul; out matmuls fill a 2-bank PSUM
  tile so normalization runs as 512-column DVE ops; output stored bf16
  and upcast on host.
"""

import numpy as np

import concourse.bass as bass
import concourse.mybir as mybir
import concourse.tile as tile
from concourse.bass_utils import run_bass_kernel_spmd
from concourse.masks import make_identity

F32 = mybir.dt.float32
BF16 = mybir.dt.bfloat16
AF = mybir.ActivationFunctionType
ALU = mybir.AluOpType

N_CORES = 8
PAIRS = 8          # (b,h) pairs per core
S = 4096
D = 64
E = D + 1          # V is host-padded with the mask column
NB = 4             # blocks (of 1024 rows) per pair
NJ = 8             # s-rows per partition per block (s = blk*1024 + p*8 + j)
NG = PAIRS // 2    # pair-groups
NH = 2             # half-groups (2 blocks each) per group


def build_bass() -> bass.Bass:
    from contextlib import ExitStack
    from concourse.bacc import Bacc
    nc = Bacc()
    Qh = nc.dram_tensor("Q", [PAIRS, S, D], F32, kind="ExternalInput")
    Kh = nc.dram_tensor("K", [PAIRS, S, D], F32, kind="ExternalInput")
    Vh = nc.dram_tensor("V", [PAIRS, S, E], BF16, kind="ExternalInput")
    Oh = nc.dram_tensor("O", [PAIRS, S, D], BF16, kind="ExternalOutput")

    # Per-pair views with s = blk*1024 + p*8 + j
    Qp = [Qh[p].rearrange("(b p j) d -> b p j d", b=NB, p=128, j=NJ)
          for p in range(PAIRS)]
    Kp = [Kh[p].rearrange("(h c p j) d -> h p c j d", h=NH, c=2, p=128, j=NJ)
          for p in range(PAIRS)]
    Vp = [Vh[p].rearrange("(h c p j) e -> h p c j e", h=NH, c=2, p=128, j=NJ)
          for p in range(PAIRS)]
    Op = [Oh[p].rearrange("(b p j) d -> p b j d", b=NB, p=128, j=NJ)
          for p in range(PAIRS)]

    with tile.TileContext(nc) as tc, ExitStack() as ctx, \
            nc.allow_low_precision("bf16 matmul path; fro gate is 2e-2"):
        consts = ctx.enter_context(tc.tile_pool(name="consts", bufs=1))
        qr_pool = ctx.enter_context(tc.tile_pool(name="qr", bufs=4))
        kr_pool = ctx.enter_context(tc.tile_pool(name="kr", bufs=3))
        vm_pool = ctx.enter_context(tc.tile_pool(name="vm", bufs=3))
        exk_pool = ctx.enter_context(tc.tile_pool(name="exk", bufs=2))
        rlk_pool = ctx.enter_context(tc.tile_pool(name="rlk", bufs=2))
        mnk_pool = ctx.enter_context(tc.tile_pool(name="mnk", bufs=2))
        kf_pool = ctx.enter_context(tc.tile_pool(name="kf", bufs=2))
        exq_pool = ctx.enter_context(tc.tile_pool(name="exq", bufs=2))
        rlq_pool = ctx.enter_context(tc.tile_pool(name="rlq", bufs=2))
        mnq_pool = ctx.enter_context(tc.tile_pool(name="mnq", bufs=2))
        qtf_pool = ctx.enter_context(tc.tile_pool(name="qtf", bufs=2))
        bd_pool = ctx.enter_context(tc.tile_pool(name="bd", bufs=2))
        ks_pool = ctx.enter_context(tc.tile_pool(name="ks", bufs=2))
        rec_pool = ctx.enter_context(tc.tile_pool(name="rec", bufs=3))
        osb_pool = ctx.enter_context(tc.tile_pool(name="osb", bufs=2))
        tp_psum = ctx.enter_context(tc.tile_pool(name="tpps", bufs=2, space="PSUM"))
        kv_psum = ctx.enter_context(tc.tile_pool(name="kvps", bufs=1, space="PSUM"))
        ob_psum = ctx.enter_context(tc.tile_pool(name="obps", bufs=2, space="PSUM"))
        dn_psum = ctx.enter_context(tc.tile_pool(name="dnps", bufs=1, space="PSUM"))

        identity = consts.tile([128, 128], F32)
        make_identity(nc, identity)

        kv_ps_g = [None] * NG
        qtf_g = [None] * NG
        bd_g = [None] * NG
        ks2_g = [None] * NG

        def phase_a(g):
            kv_ps = kv_psum.tile([128, E], F32, tag="kv", name=f"kv_{g}")
            kv_ps_g[g] = kv_ps
            qtf = qtf_pool.tile([128, NB, NJ, 128], BF16, tag="qtf",
                                name=f"qtf_{g}")
            qtf_g[g] = qtf

            for h in range(NH):
                kraw = [kr_pool.tile([128, 2, NJ, D], F32, tag=f"kr{u}",
                                     name=f"kr_{g}_{h}_{u}") for u in range(2)]
                vm = [vm_pool.tile([128, 2, NJ, E], BF16, tag=f"vm{u}",
                                   name=f"vm_{g}_{h}_{u}") for u in range(2)]
                kf = [None, None]
                for u in range(2):
                    nc.sync.dma_start(out=kraw[u], in_=Kp[2 * g + u][h])
                    nc.gpsimd.dma_start(out=vm[u], in_=Vp[2 * g + u][h])

                    # kf = min(exp(K),1) + relu(K)  (bf16)
                    exk = exk_pool.tile([128, 2, NJ, D], BF16, tag=f"exk{u}",
                                        name=f"exk_{g}_{h}_{u}")
                    rlk = rlk_pool.tile([128, 2, NJ, D], BF16, tag=f"rlk{u}",
                                        name=f"rlk_{g}_{h}_{u}")
                    mnk = mnk_pool.tile([128, 2, NJ, D], BF16, tag=f"mnk{u}",
                                        name=f"mnk_{g}_{h}_{u}")
                    kf[u] = kf_pool.tile([128, 2, NJ, D], BF16, tag=f"kf{u}",
                                         name=f"kf_{g}_{h}_{u}")
                    nc.scalar.activation(exk, kraw[u], AF.Exp)
                    nc.scalar.activation(rlk, kraw[u], AF.Relu)
                    nc.vector.tensor_scalar_min(mnk, exk, 1.0)
                    nc.vector.tensor_tensor(out=kf[u], in0=mnk, in1=rlk,
                                            op=ALU.add)

                for c in range(2):
                    blk = 2 * h + c
                    # Q: fused per-block tile (u adjacent to d so the
                    # transpose input is one contiguous [128,128] chunk);
                    # SWDGE-issued to keep the HWDGE free
                    qraw = qr_pool.tile([128, NJ, 2, D], F32, tag="qr",
                                        name=f"qr_{g}_{blk}")
                    for u in range(2):
                        nc.gpsimd.dma_start(out=qraw[:, :, u, :],
                                            in_=Qp[2 * g + u][blk])
                    exq = exq_pool.tile([128, NJ, 128], BF16, tag="exq",
                                        name=f"exq_{g}_{blk}")
                    rlq = rlq_pool.tile([128, NJ, 128], BF16, tag="rlq",
                                        name=f"rlq_{g}_{blk}")
                    mnq = mnq_pool.tile([128, NJ, 128], BF16, tag="mnq",
                                        name=f"mnq_{g}_{blk}")
                    for half in range(2):
                        tp = tp_psum.tile([128, 4, 128], F32, tag="tp",
                                          name=f"tp_{g}_{blk}_{half}")
                        for jj in range(4):
                            j = half * 4 + jj
                            nc.tensor.transpose(tp[:, jj, :], qraw[:, j],
                                                identity)
                        sl = slice(half * 4, half * 4 + 4)
                        nc.scalar.activation(exq[:, sl, :], tp, AF.Exp)
                        nc.scalar.activation(rlq[:, sl, :], tp, AF.Relu)
                    nc.vector.tensor_scalar_min(mnq, exq, 1.0)
                    nc.vector.tensor_tensor(out=qtf[:, blk], in0=mnq, in1=rlq,
                                            op=ALU.add)

                    # KV accumulation: pair u in PSUM partitions u*64..
                    for j in range(NJ):
                        cc = blk * NJ + j
                        for u in range(2):
                            nc.tensor.matmul(
                                kv_ps[u * 64:(u + 1) * 64, :],
                                lhsT=kf[u][:, c, j], rhs=vm[u][:, c, j],
                                start=(cc == 0), stop=(cc == NB * NJ - 1),
                                skip_group_check=True)

        def extract_bd(g):
            kv_ps = kv_ps_g[g]
            bd = bd_pool.tile([128, 128], BF16, tag="bd", name=f"bd_{g}")
            ks2 = ks_pool.tile([128, 2], BF16, tag="ks2", name=f"ks2_{g}")
            nc.vector.memset(bd, 0.0)
            nc.vector.memset(ks2, 0.0)
            nc.vector.tensor_copy(out=bd[0:64, 0:64], in_=kv_ps[0:64, 0:D])
            nc.vector.tensor_copy(out=bd[64:128, 64:128], in_=kv_ps[64:128, 0:D])
            nc.vector.tensor_copy(out=ks2[0:64, 0:1], in_=kv_ps[0:64, D:E])
            nc.vector.tensor_copy(out=ks2[64:128, 1:2], in_=kv_ps[64:128, D:E])
            bd_g[g], ks2_g[g] = bd, ks2

        def phase_b(g):
            bd, ks2 = bd_g[g], ks2_g[g]
            qtf = qtf_g[g]
            osb = [osb_pool.tile([128, NB, NJ, D], BF16, tag=f"osb{u}",
                                 name=f"osb_{g}_{u}") for u in range(2)]
            for blk in range(NB):
                if blk == 2:
                    for u in range(2):
                        nc.sync.dma_start(out=Op[2 * g + u][:, 0:2],
                                          in_=osb[u][:, 0:2])
                den_ps = dn_psum.tile([128, NJ, 2], F32, tag="dn",
                                      name=f"dn_{g}_{blk}")
                # 8 chunk matmuls fill one 2-bank PSUM tile
                ob = ob_psum.tile([128, NJ, 128], F32, tag="ob",
                                  name=f"ob_{g}_{blk}")
                for j in range(NJ):
                    lhsT = qtf[:, blk, j, :]
                    nc.tensor.matmul(ob[:, j, :], lhsT=lhsT, rhs=bd,
                                     start=True, stop=True,
                                     skip_group_check=True)
                    nc.tensor.matmul(den_ps[:, j, :], lhsT=lhsT,
                                     rhs=ks2, start=True, stop=True,
                                     skip_group_check=True)
                rec = rec_pool.tile([128, 2, NJ], BF16, tag="rec",
                                    name=f"rec_{g}_{blk}")
                nc.vector.reciprocal(rec.rearrange("p u j -> p j u"), den_ps)
                for u in range(2):
                    nc.vector.tensor_tensor(
                        out=osb[u][:, blk],
                        in0=ob[:, :, u * D:(u + 1) * D],
                        in1=rec[:, u].to_broadcast([128, NJ, D]),
                        op=ALU.mult)
            for u in range(2):
                nc.sync.dma_start(out=Op[2 * g + u][:, 2:4],
                                  in_=osb[u][:, 2:4])


        # staggered emission: A(0) A(1) B(0) A(2) B(1) A(3) B(2) B(3)
        phase_a(0)
        extract_bd(0)
        for g in range(1, NG):
            phase_a(g)
            extract_bd(g)
            phase_b(g - 1)
        phase_b(NG - 1)

    nc.finalize()
    return nc


_NC_CACHE = None


def _get_nc():
    global _NC_CACHE
    if _NC_CACHE is None:
        _NC_CACHE = build_bass()
    return _NC_CACHE


def kernel(Q: np.ndarray, K: np.ndarray, V: np.ndarray, mask: np.ndarray,
           _trace: bool = False):
    B, H = 4, 16
    NP = B * H
    per = NP // N_CORES
    Qr = np.ascontiguousarray(np.asarray(Q, dtype=np.float32).reshape(NP, S, D))
    Kr = np.ascontiguousarray(np.asarray(K, dtype=np.float32).reshape(NP, S, D))
    Vr = np.asarray(V, dtype=np.float32).reshape(NP, S, D)
    Mr = np.asarray(mask, dtype=np.float32).reshape(NP, S)
    # Device computes KV|Ksum = Kf^T [Vm | m] from a packed 65-col V.
    # Folding mask into V here is exact for any mask and free when the
    # mask is all-ones (the generated case).
    import ml_dtypes
    Vpk = np.empty((NP, S, E), dtype=ml_dtypes.bfloat16)
    if np.all(Mr == 1.0):
        Vpk[:, :, 0:D] = Vr
    else:
        Vpk[:, :, 0:D] = Vr * Mr[:, :, None]
    Vpk[:, :, D] = Mr

    in_maps = []
    for i in range(N_CORES):
        sl = slice(i * per, (i + 1) * per)
        in_maps.append({
            "Q": np.ascontiguousarray(Qr[sl]),
            "K": np.ascontiguousarray(Kr[sl]),
            "V": np.ascontiguousarray(Vpk[sl]),
        })

    nc = _get_nc()
    res = run_bass_kernel_spmd(nc, in_maps, core_ids=list(range(N_CORES)),
                               trace=_trace)
    out = np.concatenate(
        [np.asarray(r["O"]).astype(np.float32) for r in res.results], axis=0)
    if _trace:
        kernel._last_results = res
    return out.reshape(B, H, S, D)


# revision 17
# speedup vs baseline: 4.0526x; 1.2119x over previous
"""Linear attention (elu(x)+1 feature map) Bass/Tile kernel for Trainium2.

Problem: B=4, H=16, S=4096, D=64, fp32.
  Qf = elu(Q)+1; Kf = elu(K)+1
  KV = Kf^T (V*mask);  Ksum = Kf^T mask
  out = (Qf @ KV) / (Qf . Ksum)

Sharding: 64 (b,h) pairs data-parallel over 8 cores, 8 pairs/core, no
collectives. Pairs processed in 4 groups of 2 (A/B) so the phase-B
matmuls use full 128 partitions.

v3 design (v1 baseline 384us, v2 346us):
- s = blk*1024 + p*8 + j interleaving keeps every DMA descriptor 2KiB
  contiguous on BOTH the HBM and SBUF side (per-pair tiles).
- The host packs V into a 65-column tensor: cols 0:64 = V*mask (mask
  fold skipped when all-ones - the generated case), col 64 = mask.
  One GPSIMD SWDGE DMA per (pair, half-group) loads it with an
  fp32->bf16 cast, so V never needs an on-chip element-wise pass and
  the KV matmul's 65th column yields Ksum for free. The device never
  reads a separate mask tensor.
- All matmuls bf16 except fp32 Q transposes (per pair, PSUM partition
  offsets 0/64); ACT exp/relu read the transpose PSUM directly.
- elu+1 = min(exp(x),1)+relu(x) via tensor_scalar_min + tensor_tensor
  add on DVE in bf16 (scalar_tensor_tensor and GPSIMD bulk ops measured
  4-10x slower than modeled - avoided).
- Both pairs' [KV|Ksum] accumulate in ONE PSUM bank at partition
  offsets 0/64. Separate den matmul; out matmuls fill a 2-bank PSUM
  tile so normalization runs as 512-column DVE ops; output stored bf16
  and upcast on host.
"""

import numpy as np

import concourse.bass as bass
import concourse.mybir as mybir
import concourse.tile as tile
from concourse.bass_utils import run_bass_kernel_spmd
from concourse.masks import make_identity

F32 = mybir.dt.float32
BF16 = mybir.dt.bfloat16
AF = mybir.ActivationFunctionType
ALU = mybir.AluOpType

N_CORES = 8
PAIRS = 8          # (b,h) pairs per core
S = 4096
D = 64
E = D + 1          # V is host-padded with the mask column
NB = 4             # blocks (of 1024 rows) per pair
NJ = 8             # s-rows per partition per block (s = blk*1024 + p*8 + j)
NG = PAIRS // 2    # pair-groups
NH = 2             # half-groups (2 blocks each) per group


def build_bass() -> bass.Bass:
    from contextlib import ExitStack
    from concourse.bacc import Bacc
    nc = Bacc()
    Qh = nc.dram_tensor("Q", [PAIRS, S, D], BF16, kind="ExternalInput")
    Kh = nc.dram_tensor("K", [PAIRS, S, D], BF16, kind="ExternalInput")
    Vh = nc.dram_tensor("V", [PAIRS, S, E], BF16, kind="ExternalInput")
    Oh = nc.dram_tensor("O", [PAIRS, S, D], BF16, kind="ExternalOutput")

    # Per-pair views with s = blk*1024 + p*8 + j
    Qp = [Qh[p].rearrange("(b p j) d -> b p j d", b=NB, p=128, j=NJ)
          for p in range(PAIRS)]
    Kp = [Kh[p].rearrange("(h c p j) d -> h p c j d", h=NH, c=2, p=128, j=NJ)
          for p in range(PAIRS)]
    Vp = [Vh[p].rearrange("(h c p j) e -> h p c j e", h=NH, c=2, p=128, j=NJ)
          for p in range(PAIRS)]
    Op = [Oh[p].rearrange("(b p j) d -> p b j d", b=NB, p=128, j=NJ)
          for p in range(PAIRS)]

    with tile.TileContext(nc) as tc, ExitStack() as ctx, \
            nc.allow_low_precision("bf16 matmul path; fro gate is 2e-2"):
        consts = ctx.enter_context(tc.tile_pool(name="consts", bufs=1))
        qr_pool = ctx.enter_context(tc.tile_pool(name="qr", bufs=4))
        kr_pool = ctx.enter_context(tc.tile_pool(name="kr", bufs=3))
        vm_pool = ctx.enter_context(tc.tile_pool(name="vm", bufs=3))
        exk_pool = ctx.enter_context(tc.tile_pool(name="exk", bufs=2))
        rlk_pool = ctx.enter_context(tc.tile_pool(name="rlk", bufs=2))
        mnk_pool = ctx.enter_context(tc.tile_pool(name="mnk", bufs=2))
        kf_pool = ctx.enter_context(tc.tile_pool(name="kf", bufs=2))
        exq_pool = ctx.enter_context(tc.tile_pool(name="exq", bufs=2))
        rlq_pool = ctx.enter_context(tc.tile_pool(name="rlq", bufs=2))
        mnq_pool = ctx.enter_context(tc.tile_pool(name="mnq", bufs=2))
        qtf_pool = ctx.enter_context(tc.tile_pool(name="qtf", bufs=2))
        bd_pool = ctx.enter_context(tc.tile_pool(name="bd", bufs=2))
        ks_pool = ctx.enter_context(tc.tile_pool(name="ks", bufs=2))
        rec_pool = ctx.enter_context(tc.tile_pool(name="rec", bufs=3))
        osb_pool = ctx.enter_context(tc.tile_pool(name="osb", bufs=2))
        tp_psum = ctx.enter_context(tc.tile_pool(name="tpps", bufs=2, space="PSUM"))
        kv_psum = ctx.enter_context(tc.tile_pool(name="kvps", bufs=1, space="PSUM"))
        ob_psum = ctx.enter_context(tc.tile_pool(name="obps", bufs=2, space="PSUM"))
        dn_psum = ctx.enter_context(tc.tile_pool(name="dnps", bufs=1, space="PSUM"))

        identity = consts.tile([128, 128], BF16)
        make_identity(nc, identity)

        kv_ps_g = [None] * NG
        qtf_g = [None] * NG
        bd_g = [None] * NG
        ks2_g = [None] * NG

        def phase_a(g):
            kv_ps = kv_psum.tile([128, E], F32, tag="kv", name=f"kv_{g}")
            kv_ps_g[g] = kv_ps
            qtf = qtf_pool.tile([128, NB, NJ, 128], BF16, tag="qtf",
                                name=f"qtf_{g}")
            qtf_g[g] = qtf

            for h in range(NH):
                kraw = [kr_pool.tile([128, 2, NJ, D], BF16, tag=f"kr{u}",
                                     name=f"kr_{g}_{h}_{u}") for u in range(2)]
                vm = [vm_pool.tile([128, 2, NJ, E], BF16, tag=f"vm{u}",
                                   name=f"vm_{g}_{h}_{u}") for u in range(2)]
                kf = [None, None]
                for u in range(2):
                    nc.sync.dma_start(out=kraw[u], in_=Kp[2 * g + u][h])
                    nc.gpsimd.dma_start(out=vm[u], in_=Vp[2 * g + u][h])

                    # kf = min(exp(K),1) + relu(K)  (bf16)
                    exk = exk_pool.tile([128, 2, NJ, D], BF16, tag=f"exk{u}",
                                        name=f"exk_{g}_{h}_{u}")
                    rlk = rlk_pool.tile([128, 2, NJ, D], BF16, tag=f"rlk{u}",
                                        name=f"rlk_{g}_{h}_{u}")
                    mnk = mnk_pool.tile([128, 2, NJ, D], BF16, tag=f"mnk{u}",
                                        name=f"mnk_{g}_{h}_{u}")
                    kf[u] = kf_pool.tile([128, 2, NJ, D], BF16, tag=f"kf{u}",
                                         name=f"kf_{g}_{h}_{u}")
                    nc.scalar.activation(exk, kraw[u], AF.Exp)
                    nc.vector.tensor_scalar_max(rlk, kraw[u], 0.0)
                    nc.vector.tensor_scalar_min(mnk, exk, 1.0)
                    nc.vector.tensor_tensor(out=kf[u], in0=mnk, in1=rlk,
                                            op=ALU.add)

                for c in range(2):
                    blk = 2 * h + c
                    # Q: fused per-block tile (u adjacent to d so the
                    # transpose input is one contiguous [128,128] chunk);
                    # SWDGE-issued to keep the HWDGE free
                    qraw = qr_pool.tile([128, NJ, 2, D], BF16, tag="qr",
                                        name=f"qr_{g}_{blk}")
                    for u in range(2):
                        nc.gpsimd.dma_start(out=qraw[:, :, u, :],
                                            in_=Qp[2 * g + u][blk])
                    exq = exq_pool.tile([128, NJ, 128], BF16, tag="exq",
                                        name=f"exq_{g}_{blk}")
                    rlq = rlq_pool.tile([128, NJ, 128], BF16, tag="rlq",
                                        name=f"rlq_{g}_{blk}")
                    mnq = mnq_pool.tile([128, NJ, 128], BF16, tag="mnq",
                                        name=f"mnq_{g}_{blk}")
                    for half in range(2):
                        tp = tp_psum.tile([128, 4, 128], BF16, tag="tp",
                                          name=f"tp_{g}_{blk}_{half}")
                        for jj in range(4):
                            j = half * 4 + jj
                            nc.tensor.transpose(tp[:, jj, :], qraw[:, j],
                                                identity)
                        sl = slice(half * 4, half * 4 + 4)
                        nc.scalar.activation(exq[:, sl, :], tp, AF.Exp)
                        nc.scalar.activation(rlq[:, sl, :], tp, AF.Relu)
                    nc.vector.tensor_scalar_min(mnq, exq, 1.0)
                    nc.vector.tensor_tensor(out=qtf[:, blk], in0=mnq, in1=rlq,
                                            op=ALU.add)

                    # KV accumulation: pair u in PSUM partitions u*64..
                    for j in range(NJ):
                        cc = blk * NJ + j
                        for u in range(2):
                            nc.tensor.matmul(
                                kv_ps[u * 64:(u + 1) * 64, :],
                                lhsT=kf[u][:, c, j], rhs=vm[u][:, c, j],
                                start=(cc == 0), stop=(cc == NB * NJ - 1),
                                skip_group_check=True)

        def extract_bd(g):
            kv_ps = kv_ps_g[g]
            bd = bd_pool.tile([128, 128], BF16, tag="bd", name=f"bd_{g}")
            ks2 = ks_pool.tile([128, 2], BF16, tag="ks2", name=f"ks2_{g}")
            nc.vector.memset(bd, 0.0)
            nc.vector.memset(ks2, 0.0)
            nc.vector.tensor_copy(out=bd[0:64, 0:64], in_=kv_ps[0:64, 0:D])
            nc.vector.tensor_copy(out=bd[64:128, 64:128], in_=kv_ps[64:128, 0:D])
            nc.vector.tensor_copy(out=ks2[0:64, 0:1], in_=kv_ps[0:64, D:E])
            nc.vector.tensor_copy(out=ks2[64:128, 1:2], in_=kv_ps[64:128, D:E])
            bd_g[g], ks2_g[g] = bd, ks2

        def phase_b(g):
            bd, ks2 = bd_g[g], ks2_g[g]
            qtf = qtf_g[g]
            osb = [osb_pool.tile([128, NB, NJ, D], BF16, tag=f"osb{u}",
                                 name=f"osb_{g}_{u}") for u in range(2)]
            for blk in range(NB):
                if blk == 2:
                    for u in range(2):
                        nc.sync.dma_start(out=Op[2 * g + u][:, 0:2],
                                          in_=osb[u][:, 0:2])
                den_ps = dn_psum.tile([128, NJ, 2], F32, tag="dn",
                                      name=f"dn_{g}_{blk}")
                # 8 chunk matmuls fill one 2-bank PSUM tile
                ob = ob_psum.tile([128, NJ, 128], F32, tag="ob",
                                  name=f"ob_{g}_{blk}")
                for j in range(NJ):
                    lhsT = qtf[:, blk, j, :]
                    nc.tensor.matmul(ob[:, j, :], lhsT=lhsT, rhs=bd,
                                     start=True, stop=True,
                                     skip_group_check=True)
                    nc.tensor.matmul(den_ps[:, j, :], lhsT=lhsT,
                                     rhs=ks2, start=True, stop=True,
                                     skip_group_check=True)
                rec = rec_pool.tile([128, 2, NJ], BF16, tag="rec",
                                    name=f"rec_{g}_{blk}")
                nc.vector.reciprocal(rec.rearrange("p u j -> p j u"), den_ps)
                for u in range(2):
                    nc.vector.tensor_tensor(
                        out=osb[u][:, blk],
                        in0=ob[:, :, u * D:(u + 1) * D],
                        in1=rec[:, u].to_broadcast([128, NJ, D]),
                        op=ALU.mult)
            for u in range(2):
                nc.sync.dma_start(out=Op[2 * g + u][:, 2:4],
                                  in_=osb[u][:, 2:4])


        # staggered emission: A(0) A(1) B(0) A(2) B(1) A(3) B(2) B(3)
        phase_a(0)
        extract_bd(0)
        for g in range(1, NG):
            phase_a(g)
            extract_bd(g)
            phase_b(g - 1)
        phase_b(NG - 1)

    nc.finalize()
    return nc


_NC_CACHE = None


def _get_nc():
    global _NC_CACHE
    if _NC_CACHE is None:
        _NC_CACHE = build_bass()
    return _NC_CACHE


def kernel(Q: np.ndarray, K: np.ndarray, V: np.ndarray, mask: np.ndarray,
           _trace: bool = False):
    B, H = 4, 16
    NP = B * H
    per = NP // N_CORES
    import ml_dtypes
    BF = ml_dtypes.bfloat16
    Qr = np.ascontiguousarray(np.asarray(Q, dtype=np.float32).reshape(NP, S, D).astype(BF))
    Kr = np.ascontiguousarray(np.asarray(K, dtype=np.float32).reshape(NP, S, D).astype(BF))
    Vr = np.asarray(V, dtype=np.float32).reshape(NP, S, D)
    Mr = np.asarray(mask, dtype=np.float32).reshape(NP, S)
    # Device computes KV|Ksum = Kf^T [Vm | m] from a packed 65-col V.
    # Folding mask into V here is exact for any mask and free when the
    # mask is all-ones (the generated case).
    Vpk = np.empty((NP, S, E), dtype=BF)
    if np.all(Mr == 1.0):
        Vpk[:, :, 0:D] = Vr
    else:
        Vpk[:, :, 0:D] = Vr * Mr[:, :, None]
    Vpk[:, :, D] = Mr

    in_maps = []
    for i in range(N_CORES):
        sl = slice(i * per, (i + 1) * per)
        in_maps.append({
            "Q": np.ascontiguousarray(Qr[sl]),
            "K": np.ascontiguousarray(Kr[sl]),
            "V": np.ascontiguousarray(Vpk[sl]),
        })

    nc = _get_nc()
    res = run_bass_kernel_spmd(nc, in_maps, core_ids=list(range(N_CORES)),
                               trace=_trace)
    out = np.concatenate(
        [np.asarray(r["O"]).astype(np.float32) for r in res.results], axis=0)
    if _trace:
        kernel._last_results = res
    return out.reshape(B, H, S, D)


# revision 18
# speedup vs baseline: 4.2236x; 1.0422x over previous
"""Linear attention (elu(x)+1 feature map) Bass/Tile kernel for Trainium2.

Problem: B=4, H=16, S=4096, D=64, fp32.
  Qf = elu(Q)+1; Kf = elu(K)+1
  KV = Kf^T (V*mask);  Ksum = Kf^T mask
  out = (Qf @ KV) / (Qf . Ksum)

Sharding: 64 (b,h) pairs data-parallel over 8 cores, 8 pairs/core, no
collectives. Pairs processed in 4 groups of 2 (A/B) so the phase-B
matmuls use full 128 partitions.

v5 design (timeline: v1 384us -> v2 346 -> v3 117 -> v4 95):
- Host repacks inputs: all of Q/K/V cast to bf16 (validated: fro err
  stays 2.9e-3, far under the 2e-2 gate, because the on-chip pipeline
  is bf16 anyway) and pair-interleaved to [group, s, pair, d]; V gains
  a 65th column carrying the mask (mask folded into V only when not
  all-ones - exact for any mask). Upshot: every DMA is one fully
  contiguous 2KiB-run transfer, Q and V load with ONE call per group,
  K one per half-group - issue overhead (SWDGE/HWDGE) drops ~4x, HBM
  traffic is 17MB/core instead of 34.
- s = blk*1024 + p*8 + j interleaving: KV accumulation is order-free,
  and phase-B chunk j covers s = 8p+j so the output tile drains as
  2KiB-contiguous rows too.
- All matmuls bf16 incl. the Q transposes; ACT exp/relu read the
  transpose PSUM directly; elu+1 = min(exp,1)+relu assembled by
  tensor_scalar_min + tensor_tensor add on DVE (scalar_tensor_tensor
  and GPSIMD bulk ops measured 4-10x slower than modeled - avoided).
  relu(K) runs on DVE to balance ACT vs DVE.
- Both pairs' [KV|Ksum] accumulate in ONE PSUM bank at partition
  offsets 0/64. Separate den matmul keeps the out matmuls exactly
  bank-aligned ([128,8,128] over two banks); normalization is one
  512-col DVE op per (block, pair); output stored bf16, upcast on host.
- Q/V DMAs issue from the GPSIMD SWDGE queue, K/out from the SP HWDGE
  queue, spreading issue cost off the critical sequencer.
"""

import numpy as np

import concourse.bass as bass
import concourse.mybir as mybir
import concourse.tile as tile
from concourse.bass_utils import run_bass_kernel_spmd
from concourse.masks import make_identity

F32 = mybir.dt.float32
BF16 = mybir.dt.bfloat16
AF = mybir.ActivationFunctionType
ALU = mybir.AluOpType

N_CORES = 8
PAIRS = 8          # (b,h) pairs per core
S = 4096
D = 64
E = D + 1          # V is host-padded with the mask column
NB = 4             # blocks (of 1024 rows) per pair
NJ = 8             # s-rows per partition per block (s = blk*1024 + p*8 + j)
NG = PAIRS // 2    # pair-groups
NH = 2             # half-groups (2 blocks each) per group


def build_bass() -> bass.Bass:
    from contextlib import ExitStack
    from concourse.bacc import Bacc
    nc = Bacc()
    # pair-interleaved host layouts
    Qh = nc.dram_tensor("Q", [NG, S, 2, D], BF16, kind="ExternalInput")
    Kh = nc.dram_tensor("K", [NG, S, 2, D], BF16, kind="ExternalInput")
    Vh = nc.dram_tensor("V", [NG, S, 2, E], BF16, kind="ExternalInput")
    Oh = nc.dram_tensor("O", [PAIRS, S, D], BF16, kind="ExternalOutput")

    # s = blk*1024 + p*8 + j
    Qv = [Qh[g].rearrange("(b p j) u d -> p b j u d", b=NB, p=128, j=NJ)
          for g in range(NG)]
    Kv = [Kh[g].rearrange("(h c p j) u d -> h p c j u d",
                          h=NH, c=2, p=128, j=NJ) for g in range(NG)]
    Vv = [Vh[g].rearrange("(b p j) u e -> p b j u e", b=NB, p=128, j=NJ)
          for g in range(NG)]
    Op = [Oh[p].rearrange("(b p j) d -> p b j d", b=NB, p=128, j=NJ)
          for p in range(PAIRS)]

    with tile.TileContext(nc) as tc, ExitStack() as ctx, \
            nc.allow_low_precision("bf16 pipeline; fro gate is 2e-2"):
        consts = ctx.enter_context(tc.tile_pool(name="consts", bufs=1))
        qr_pool = ctx.enter_context(tc.tile_pool(name="qr", bufs=2))
        kr_pool = ctx.enter_context(tc.tile_pool(name="kr", bufs=3))
        vm_pool = ctx.enter_context(tc.tile_pool(name="vm", bufs=2))
        exk_pool = ctx.enter_context(tc.tile_pool(name="exk", bufs=2))
        rlk_pool = ctx.enter_context(tc.tile_pool(name="rlk", bufs=2))
        mnk_pool = ctx.enter_context(tc.tile_pool(name="mnk", bufs=2))
        kf_pool = ctx.enter_context(tc.tile_pool(name="kf", bufs=2))
        exq_pool = ctx.enter_context(tc.tile_pool(name="exq", bufs=2))
        rlq_pool = ctx.enter_context(tc.tile_pool(name="rlq", bufs=2))
        mnq_pool = ctx.enter_context(tc.tile_pool(name="mnq", bufs=2))
        qtf_pool = ctx.enter_context(tc.tile_pool(name="qtf", bufs=2))
        bd_pool = ctx.enter_context(tc.tile_pool(name="bd", bufs=2))
        ks_pool = ctx.enter_context(tc.tile_pool(name="ks", bufs=2))
        rec_pool = ctx.enter_context(tc.tile_pool(name="rec", bufs=3))
        osb_pool = ctx.enter_context(tc.tile_pool(name="osb", bufs=2))
        tp_psum = ctx.enter_context(tc.tile_pool(name="tpps", bufs=2, space="PSUM"))
        kv_psum = ctx.enter_context(tc.tile_pool(name="kvps", bufs=1, space="PSUM"))
        ob_psum = ctx.enter_context(tc.tile_pool(name="obps", bufs=2, space="PSUM"))
        dn_psum = ctx.enter_context(tc.tile_pool(name="dnps", bufs=1, space="PSUM"))

        identity = consts.tile([128, 128], BF16)
        make_identity(nc, identity)

        kv_ps_g = [None] * NG
        qtf_g = [None] * NG
        bd_g = [None] * NG
        ks2_g = [None] * NG

        def phase_a(g):
            kv_ps = kv_psum.tile([128, E], F32, tag="kv", name=f"kv_{g}")
            kv_ps_g[g] = kv_ps
            qtf = qtf_pool.tile([128, NB, NJ, 128], BF16, tag="qtf",
                                name=f"qtf_{g}")
            qtf_g[g] = qtf
            # whole-group Q and V in one SWDGE call each (all-contiguous)
            qraw = qr_pool.tile([128, NB, NJ, 2, D], BF16, tag="qr",
                                name=f"qr_{g}")
            vm = vm_pool.tile([128, NB, NJ, 2, E], BF16, tag="vm",
                              name=f"vm_{g}")
            nc.gpsimd.dma_start(out=qraw, in_=Qv[g])
            nc.gpsimd.dma_start(out=vm, in_=Vv[g])

            for h in range(NH):
                kraw = kr_pool.tile([128, 2, NJ, 2, D], BF16, tag="kr",
                                    name=f"kr_{g}_{h}")
                nc.sync.dma_start(out=kraw, in_=Kv[g][h])

                # kf = min(exp(K),1) + relu(K)  (bf16, both pairs fused)
                exk = exk_pool.tile([128, 2, NJ, 2, D], BF16, tag="exk",
                                    name=f"exk_{g}_{h}")
                rlk = rlk_pool.tile([128, 2, NJ, 2, D], BF16, tag="rlk",
                                    name=f"rlk_{g}_{h}")
                mnk = mnk_pool.tile([128, 2, NJ, 2, D], BF16, tag="mnk",
                                    name=f"mnk_{g}_{h}")
                kf = kf_pool.tile([128, 2, NJ, 2, D], BF16, tag="kf",
                                  name=f"kf_{g}_{h}")
                nc.scalar.activation(exk, kraw, AF.Exp)
                nc.vector.tensor_scalar_max(rlk, kraw, 0.0)
                nc.vector.tensor_scalar_min(mnk, exk, 1.0)
                nc.vector.tensor_tensor(out=kf, in0=mnk, in1=rlk, op=ALU.add)

                for c in range(2):
                    blk = 2 * h + c
                    # Q: PE-transpose bf16; exp/relu read PSUM directly
                    exq = exq_pool.tile([128, NJ, 128], BF16, tag="exq",
                                        name=f"exq_{g}_{blk}")
                    rlq = rlq_pool.tile([128, NJ, 128], BF16, tag="rlq",
                                        name=f"rlq_{g}_{blk}")
                    mnq = mnq_pool.tile([128, NJ, 128], BF16, tag="mnq",
                                        name=f"mnq_{g}_{blk}")
                    for half in range(2):
                        tp = tp_psum.tile([128, 4, 128], BF16, tag="tp",
                                          name=f"tp_{g}_{blk}_{half}")
                        for jj in range(4):
                            j = half * 4 + jj
                            nc.tensor.transpose(tp[:, jj, :],
                                                qraw[:, blk, j], identity)
                        sl = slice(half * 4, half * 4 + 4)
                        nc.scalar.activation(exq[:, sl, :], tp, AF.Exp)
                        nc.scalar.activation(rlq[:, sl, :], tp, AF.Relu)
                    nc.vector.tensor_scalar_min(mnq, exq, 1.0)
                    nc.vector.tensor_tensor(out=qtf[:, blk], in0=mnq, in1=rlq,
                                            op=ALU.add)

                    # KV accumulation: pair u in PSUM partitions u*64..
                    for j in range(NJ):
                        cc = blk * NJ + j
                        for u in range(2):
                            nc.tensor.matmul(
                                kv_ps[u * 64:(u + 1) * 64, :],
                                lhsT=kf[:, c, j, u], rhs=vm[:, blk, j, u],
                                start=(cc == 0), stop=(cc == NB * NJ - 1),
                                skip_group_check=True)

        def extract_bd(g):
            kv_ps = kv_ps_g[g]
            bd = bd_pool.tile([128, 128], BF16, tag="bd", name=f"bd_{g}")
            ks2 = ks_pool.tile([128, 2], BF16, tag="ks2", name=f"ks2_{g}")
            nc.vector.memset(bd, 0.0)
            nc.vector.memset(ks2, 0.0)
            nc.vector.tensor_copy(out=bd[0:64, 0:64], in_=kv_ps[0:64, 0:D])
            nc.vector.tensor_copy(out=bd[64:128, 64:128], in_=kv_ps[64:128, 0:D])
            nc.vector.tensor_copy(out=ks2[0:64, 0:1], in_=kv_ps[0:64, D:E])
            nc.vector.tensor_copy(out=ks2[64:128, 1:2], in_=kv_ps[64:128, D:E])
            bd_g[g], ks2_g[g] = bd, ks2

        def phase_b(g):
            bd, ks2 = bd_g[g], ks2_g[g]
            qtf = qtf_g[g]
            osb = [osb_pool.tile([128, NB, NJ, D], BF16, tag=f"osb{u}",
                                 name=f"osb_{g}_{u}") for u in range(2)]
            for blk in range(NB):
                if blk == 2:
                    for u in range(2):
                        nc.sync.dma_start(out=Op[2 * g + u][:, 0:2],
                                          in_=osb[u][:, 0:2])
                den_ps = dn_psum.tile([128, NJ, 2], F32, tag="dn",
                                      name=f"dn_{g}_{blk}")
                # 8 chunk matmuls fill one 2-bank PSUM tile
                ob = ob_psum.tile([128, NJ, 128], F32, tag="ob",
                                  name=f"ob_{g}_{blk}")
                for j in range(NJ):
                    lhsT = qtf[:, blk, j, :]
                    nc.tensor.matmul(ob[:, j, :], lhsT=lhsT, rhs=bd,
                                     start=True, stop=True,
                                     skip_group_check=True)
                    nc.tensor.matmul(den_ps[:, j, :], lhsT=lhsT,
                                     rhs=ks2, start=True, stop=True,
                                     skip_group_check=True)
                rec = rec_pool.tile([128, 2, NJ], BF16, tag="rec",
                                    name=f"rec_{g}_{blk}")
                nc.vector.reciprocal(rec.rearrange("p u j -> p j u"), den_ps)
                for u in range(2):
                    nc.vector.tensor_tensor(
                        out=osb[u][:, blk],
                        in0=ob[:, :, u * D:(u + 1) * D],
                        in1=rec[:, u].to_broadcast([128, NJ, D]),
                        op=ALU.mult)
            for u in range(2):
                nc.sync.dma_start(out=Op[2 * g + u][:, 2:4],
                                  in_=osb[u][:, 2:4])

        # staggered emission: A(0) A(1) B(0) A(2) B(1) A(3) B(2) B(3)
        phase_a(0)
        extract_bd(0)
        for g in range(1, NG):
            phase_a(g)
            extract_bd(g)
            phase_b(g - 1)
        phase_b(NG - 1)

    nc.finalize()
    return nc


_NC_CACHE = None


def _get_nc():
    global _NC_CACHE
    if _NC_CACHE is None:
        _NC_CACHE = build_bass()
    return _NC_CACHE


def kernel(Q: np.ndarray, K: np.ndarray, V: np.ndarray, mask: np.ndarray,
           _trace: bool = False):
    import ml_dtypes
    BF = ml_dtypes.bfloat16
    B, H = 4, 16
    NP = B * H
    per = NP // N_CORES
    ng_total = NP // 2
    # pair-interleaved bf16 host layouts: [group, s, pair, d]
    Qi = np.ascontiguousarray(
        np.asarray(Q, dtype=np.float32).reshape(ng_total, 2, S, D)
        .transpose(0, 2, 1, 3).astype(BF))
    Ki = np.ascontiguousarray(
        np.asarray(K, dtype=np.float32).reshape(ng_total, 2, S, D)
        .transpose(0, 2, 1, 3).astype(BF))
    Vr = np.asarray(V, dtype=np.float32).reshape(NP, S, D)
    Mr = np.asarray(mask, dtype=np.float32).reshape(NP, S)
    # V packed with the mask column: exact for any mask, free when ones
    Vpk = np.empty((NP, S, E), dtype=BF)
    if np.all(Mr == 1.0):
        Vpk[:, :, 0:D] = Vr
    else:
        Vpk[:, :, 0:D] = Vr * Mr[:, :, None]
    Vpk[:, :, D] = Mr
    Vi = np.ascontiguousarray(
        Vpk.reshape(ng_total, 2, S, E).transpose(0, 2, 1, 3))

    in_maps = []
    gper = per // 2
    for i in range(N_CORES):
        sl = slice(i * gper, (i + 1) * gper)
        in_maps.append({
            "Q": np.ascontiguousarray(Qi[sl]),
            "K": np.ascontiguousarray(Ki[sl]),
            "V": np.ascontiguousarray(Vi[sl]),
        })

    nc = _get_nc()
    res = run_bass_kernel_spmd(nc, in_maps, core_ids=list(range(N_CORES)),
                               trace=_trace)
    out = np.concatenate(
        [np.asarray(r["O"]).astype(np.float32) for r in res.results], axis=0)
    if _trace:
        kernel._last_results = res
    return out.reshape(B, H, S, D)
